# revision 6
# baseline (speedup 1.0000x reference)
"""v31 Trainium2 Bass kernel for an attention-style graph convolution (GAT).

Same staircase P/Q/band math as v29, restructured for DMA/engine efficiency:
  - m8 stored DRAM-contiguous per partition (P, NJ*MI); streamed as 17
    transfers (first/last chunk single, rest chunk-pairs, 4KB rows),
    interleaved with gq/gp slab loads across the sync+scalar HW DGE queues
    in exact consumption order, all pre-issued so the DMA subsystem runs at
    peak with no buffer-recycling stalls.
  - Everything is SBUF-resident (m8 64KB/partition); no mid-run reloads.
  - Per-block P->Q transition scales stay on DVE but are emitted before the
    chunk's matmuls, and transitioning blocks are matmul'd last within the
    chunk, hiding the scale latency behind other blocks' matmuls.
  - Output written as one contiguous [128, NIT*GW] bf16 DMA (was 2048
    small packets taking ~5us).
Host sums j-half core pairs, un-permutes rows, divides, applies elu.
"""

import ml_dtypes
import numpy as np

import concourse.bacc as bacc
import concourse.bass as bass
import concourse.mybir as mybir
import concourse.tile as tile
from concourse import bass_utils

F32 = mybir.dt.float32
BF16 = mybir.dt.bfloat16
FP16 = mybir.dt.float16
FP8 = mybir.dt.float8e4
OP = mybir.AluOpType

N = 8192
K = 256
F = 128
ALPHA = 0.2
NCORES = 8
MI = 2048
MJ = 4096
P = 128
NJ = MJ // P      # 32 j-chunks
NIT = MI // P     # 16 i-blocks
LAG = 4
GW = F + 1
SLAB = 8          # gq/gp slab = 8 chunks


def _broadcast_ap(row_ap, nparts):
    return bass.AP(
        tensor=row_ap.tensor,
        offset=row_ap.offset,
        ap=[[0, nparts]] + [list(d) for d in row_ap.ap],
    )


def build_program(kq, kp):
    BW = max(1, max(p - q for p, q in zip(kp, kq)))
    nc = bacc.Bacc("TRN2", target_bir_lowering=False)

    m8_d = nc.dram_tensor("m8", (P, NJ * MI), FP8, kind="ExternalInput")
    gq_d = nc.dram_tensor("gq", (P, NJ * GW), FP16, kind="ExternalInput")
    gp_d = nc.dram_tensor("gp", (P, NJ * GW), FP16, kind="ExternalInput")
    es1b_d = nc.dram_tensor("es1b", (1, MI), FP16, kind="ExternalInput")
    es1bt_d = nc.dram_tensor("es1bt", (P, NIT), F32, kind="ExternalInput")
    es2m_d = nc.dram_tensor("es2m", (P, NJ), F32, kind="ExternalInput")
    out_d = nc.dram_tensor("out", (P, NIT * GW), BF16, kind="ExternalOutput")

    # per-block count of P-chunks (suffix in c); 0 -> no scale needed
    pP = [sum(1 for c in range(NJ) if kp[c] <= it) for it in range(NIT)]

    # m8 transfers: [31], [30,29], [28,27], ..., [2,1], [0]
    m8_groups = [[NJ - 1]]
    c = NJ - 2
    while c >= 1:
        m8_groups.append([c, c - 1])
        c -= 2
    m8_groups.append([0])

    with tile.TileContext(nc) as tc:
        with (
            tc.tile_pool(name="consts", bufs=1) as consts,
            tc.tile_pool(name="adj", bufs=1) as adjp,
            tc.tile_pool(name="gqp", bufs=1) as gqp,
            tc.tile_pool(name="gpp", bufs=1) as gpp,
            tc.tile_pool(name="up", bufs=3) as up,
            tc.tile_pool(name="ntp", bufs=5) as ntp,
            tc.tile_pool(name="outp", bufs=1) as outp,
            tc.tile_pool(name="ps_acc", bufs=1, space="PSUM") as ps_acc,
        ):
            es2m = consts.tile([P, NJ], F32, tag="es2m")
            es1bt = consts.tile([P, NIT], F32, tag="es1bt")
            es1b = consts.tile([P, MI], FP16, tag="es1b")

            nslab = NJ // SLAB
            gqt = [
                gqp.tile([P, SLAB * GW], FP16, tag=f"gq{s}", name=f"gq{s}")
                for s in range(nslab)
            ]
            gpt = [
                gpp.tile([P, SLAB * GW], FP16, tag=f"gp{s}", name=f"gp{s}")
                for s in range(nslab)
            ]
            m8t = [
                adjp.tile(
                    [P, len(g) * MI], FP8, tag=f"m8_{gi}", name=f"m8_{gi}"
                )
                for gi, g in enumerate(m8_groups)
            ]

            accs = [
                ps_acc.tile([P, 512], F32, tag=f"acc{b}", name=f"acc{b}")
                for b in range(8)
            ]

            def acc_slice(it):
                return accs[it // 2][:, (it % 2) * 256 : (it % 2) * 256 + GW]

            # chunk -> (transfer index, offset-in-transfer)
            c2t = {}
            for gi, g in enumerate(m8_groups):
                for ofs, cc in enumerate(sorted(g)):
                    c2t[cc] = (gi, ofs)

            def dma_m8(eng, gi):
                g = m8_groups[gi]
                lo = min(g)
                eng.dma_start(
                    out=m8t[gi][:],
                    in_=m8_d[:, lo * MI : (lo + len(g)) * MI],
                )

            def dma_slab(eng, t, d, s):
                eng.dma_start(
                    out=t[s][:], in_=d[:, s * SLAB * GW : (s + 1) * SLAB * GW]
                )

            # ---- DMA issue phase: everything up front, in consumption order
            # es1b broadcast split: first w1 blocks cover bands of c>=26
            w1 = max(max(kp[c] for c in range(NJ - 6, NJ)), 1)
            S, A = nc.sync, nc.scalar
            A.dma_start(out=es2m[:], in_=es2m_d[:, :])
            A.dma_start(out=es1bt[:], in_=es1bt_d[:, :])
            dma_m8(S, 0)                      # c31
            dma_slab(A, gqt, gq_d, 3)
            dma_slab(A, gpt, gp_d, 3)
            dma_m8(S, 1)                      # c30,29
            A.dma_start(
                out=es1b[:, : w1 * P],
                in_=_broadcast_ap(es1b_d[:, : w1 * P], P),
            )
            dma_m8(S, 2)                      # c28,27
            S.dma_start(
                out=es1b[:, w1 * P :],
                in_=_broadcast_ap(es1b_d[:, w1 * P :], P),
            )
            dma_slab(A, gqt, gq_d, 2)
            dma_m8(S, 3)                      # c26,25
            dma_slab(A, gpt, gp_d, 2)
            dma_m8(S, 4)                      # c24,23
            dma_m8(S, 5)                      # c22,21
            dma_m8(A, 6)                      # c20,19
            dma_slab(A, gqt, gq_d, 1)
            dma_m8(S, 7)                      # c18,17
            dma_slab(A, gpt, gp_d, 1)
            dma_m8(S, 8)                      # c16,15
            dma_m8(A, 9)                      # c14,13
            dma_m8(S, 10)                     # c12,11
            dma_slab(A, gqt, gq_d, 0)
            dma_slab(A, gpt, gp_d, 0)
            dma_m8(S, 11)                     # c10,9
            dma_m8(A, 12)                     # c8,7
            dma_m8(S, 13)                     # c6,5
            dma_m8(A, 14)                     # c4,3
            dma_m8(S, 15)                     # c2,1
            dma_m8(S, 16)                     # c0

            def m8_stat(c, it):
                gi, ofs = c2t[c]
                return m8t[gi][:, ofs * MI + it * P : ofs * MI + (it + 1) * P]

            def gq_mov(c):
                return gqt[c // SLAB][:, (c % SLAB) * GW : (c % SLAB) * GW + GW]

            def gp_mov(c):
                return gpt[c // SLAB][:, (c % SLAB) * GW : (c % SLAB) * GW + GW]

            pend = []
            scaled = [False] * NIT
            band_tiles = {}

            def phase_a(c):
                bw = kp[c] - kq[c]
                if bw > 0:
                    w = bw * P
                    u_t = up.tile([P, BW * P], FP16, tag="u_t")
                    nc.vector.tensor_scalar(
                        out=u_t[:, :w],
                        in0=es1b[:, kq[c] * P : kp[c] * P],
                        scalar1=es2m[:, c : c + 1],
                        scalar2=1.0,
                        op0=OP.mult,
                        op1=OP.max,
                    )
                    n_t = ntp.tile([P, BW * P], FP16, tag="n_t")
                    gi, ofs = c2t[c]
                    nc.vector.tensor_tensor(
                        out=n_t[:, :w],
                        in0=u_t[:, :w],
                        in1=m8t[gi][
                            :, ofs * MI + kq[c] * P : ofs * MI + kp[c] * P
                        ],
                        op=OP.mult,
                    )
                    band_tiles[c] = n_t
                pend.append(c)

            res = outp.tile([P, NIT * GW], BF16, tag="res")

            def scale_acc(it):
                nc.vector.tensor_scalar(
                    out=acc_slice(it),
                    in0=acc_slice(it),
                    scalar1=es1bt[:, it : it + 1],
                    scalar2=None,
                    op0=OP.mult,
                )
                scaled[it] = True

            def phase_c():
                c = pend.pop(0)
                n_t = band_tiles.pop(c, None)
                last = c == 0
                trans = [
                    it
                    for it in range(NIT)
                    if it < kp[c] and pP[it] > 0 and not scaled[it]
                ]
                for it in trans:
                    scale_acc(it)
                # banks ordered: plain P/Q first, band-containing, then
                # transitioning last (hides scale/n_t latency); c=0 keeps
                # natural order so copies/out-DMA pipeline per half.
                def bank_key(b):
                    its = (2 * b, 2 * b + 1)
                    return (
                        any(it in trans for it in its),
                        any(kq[c] <= it < kp[c] for it in its),
                        b,
                    )

                banks = (
                    list(range(8)) if last else sorted(range(8), key=bank_key)
                )
                for b in banks:
                    for it in (2 * b, 2 * b + 1):
                        if it >= kp[c]:
                            stat = m8_stat(c, it)
                            mov = gp_mov(c)
                        elif it < kq[c]:
                            stat = m8_stat(c, it)
                            mov = gq_mov(c)
                        else:
                            stat = n_t[
                                :, (it - kq[c]) * P : (it - kq[c] + 1) * P
                            ]
                            mov = gq_mov(c)
                        nc.tensor.matmul(
                            acc_slice(it),
                            stat,
                            mov,
                            start=(c == NJ - 1 and it % 2 == 0),
                            stop=last,
                            skip_group_check=True,
                        )
                    if last:
                        for it in (2 * b, 2 * b + 1):
                            if pP[it] > 0 and not scaled[it]:
                                scale_acc(it)
                            dst = res[:, it * GW : (it + 1) * GW]
                            if it % 2 == 0:
                                nc.vector.tensor_copy(dst, acc_slice(it))
                            else:
                                nc.scalar.copy(dst, acc_slice(it))
                        if b == 3:
                            nc.sync.dma_start(
                                out=out_d[:, : 8 * GW], in_=res[:, : 8 * GW]
                            )
                        elif b == 7:
                            nc.scalar.dma_start(
                                out=out_d[:, 8 * GW :], in_=res[:, 8 * GW :]
                            )

            order = list(range(NJ - 1, -1, -1))
            for idx, c in enumerate(order):
                phase_a(c)
                if idx >= LAG:
                    phase_c()
            while pend:
                phase_c()

    nc.compile()
    return nc


def host_prepare(x, adj, W, a):
    h64 = x.astype(np.float64) @ W.astype(np.float64)
    s1 = h64 @ a[:F, 0].astype(np.float64)
    s2 = h64 @ a[F:, 0].astype(np.float64)
    es2a = np.exp(ALPHA * s2)
    es2m = np.exp((1.0 - ALPHA) * s2)
    es2f = np.exp(s2)
    g2 = np.empty((N, GW), np.float64)
    g2[:, :F] = h64
    g2[:, F] = 1.0
    gq = (g2 * es2a[:, None]).astype(np.float16)
    gp = (g2 * es2f[:, None]).astype(np.float16)
    es1b16 = np.exp((1.0 - ALPHA) * s1).astype(np.float16)

    isort = np.argsort(s1, kind="stable")
    ilists = [isort[sl::4] for sl in range(4)]
    jsort = np.argsort(s2, kind="stable")
    jlists = [jsort[h::2] for h in range(2)]

    maskT = adj.T > 0

    kq_all, kp_all = [], []
    for h in range(2):
        es2m_h = es2m[jlists[h]].astype(np.float32)
        cmax = es2m_h.reshape(NJ, P).max(axis=1)
        cmin = es2m_h.reshape(NJ, P).min(axis=1)
        for sl in range(4):
            e1 = es1b16[ilists[sl]].astype(np.float32).reshape(NIT, P)
            bmax = e1.max(axis=1)
            bmin = e1.min(axis=1)
            kq_all.append((bmax[None, :] * cmax[:, None] <= 1.0).sum(axis=1))
            # P-suffix count: blocks with bmin * es2m_chunk_min >= 1
            cnt = (bmin[None, :] * cmin[:, None] >= 1.0).sum(axis=1)
            kp_all.append(NIT - cnt)
    kq = np.minimum.reduce(kq_all).astype(int)
    kp = np.maximum.reduce(kp_all).astype(int)
    kp = np.maximum(kp, kq)  # band must be non-negative
    kq_l, kp_l = kq.tolist(), kp.tolist()

    in_maps = []
    for c in range(NCORES):
        sl = c % 4
        h = c // 4
        il, jl = ilists[sl], jlists[h]
        mT = maskT[np.ix_(jl, il)]
        # (P, NJ*MI): row p holds chunk-contiguous mask data
        m8 = np.ascontiguousarray(
            mT.reshape(NJ, P, MI).transpose(1, 0, 2).reshape(P, NJ * MI)
        ).astype(ml_dtypes.float8_e4m3)

        gqh = np.ascontiguousarray(
            gq[jl].reshape(NJ, P, GW).transpose(1, 0, 2).reshape(P, NJ * GW)
        )
        gph = np.ascontiguousarray(
            gp[jl].reshape(NJ, P, GW).transpose(1, 0, 2).reshape(P, NJ * GW)
        )
        es2mh = np.ascontiguousarray(es2m[jl].reshape(NJ, P).T.astype(np.float32))
        es1bth = np.ascontiguousarray(
            es1b16[il].astype(np.float32).reshape(NIT, P).T
        )
        in_maps.append(
            {
                "m8": m8,
                "gq": gqh,
                "gp": gph,
                "es1b": es1b16[il].reshape(1, MI),
                "es1bt": es1bth,
                "es2m": es2mh,
            }
        )
    return in_maps, kq_l, kp_l, ilists


_NC_CACHE = {}


def kernel(x, adj, W, a, _trace=False):
    x = np.asarray(x)
    adj = np.asarray(adj)
    W = np.asarray(W)
    a = np.asarray(a)

    in_maps, kq, kp, ilists = host_prepare(x, adj, W, a)
    key = (tuple(kq), tuple(kp))
    if key not in _NC_CACHE:
        _NC_CACHE.clear()
        _NC_CACHE[key] = build_program(kq, kp)
    nc = _NC_CACHE[key]
    res = bass_utils.run_bass_kernel_spmd(
        nc, in_maps, core_ids=list(range(NCORES)), trace=_trace
    )
    nd = np.empty((N, GW), np.float32)
    for sl in range(4):
        a0 = np.asarray(res.results[sl]["out"]).astype(np.float32)
        a1 = np.asarray(res.results[sl + 4]["out"]).astype(np.float32)
        both = (a0 + a1).reshape(P, NIT, GW).transpose(1, 0, 2).reshape(MI, GW)
        nd[ilists[sl]] = both
    hp = nd[:, :F] / nd[:, F : F + 1]
    out = np.where(hp > 0, hp, np.expm1(np.minimum(hp, 0.0))).astype(np.float32)
    if _trace:
        return out, res
    return out


# revision 7
# speedup vs baseline: 1.0280x; 1.0280x over previous
"""v33 Trainium2 Bass kernel for an attention-style graph convolution (GAT).

Same staircase P/Q/band math as v29; DMA schedule rebuilt around the measured
per-transfer descriptor floor (~1.3-3us for any [128,*] SBUF transfer):
  - m8 DRAM-contiguous per partition (P, NJ*MI); 12 transfers (pairs early
    for latency, quads later for efficiency) interleaved across the sync and
    scalar HW DGE queues in exact consumption order, all pre-issued.
  - gq|gp packed per 8-chunk slab into one DRAM tensor -> 4 transfers.
  - es2m/es1bt/es1b[0:512] packed into one byte-blob "meta" transfer
    (bitcast views); remaining es1b materialized host-side (no stride-0
    broadcast DMA).
  - Per-block P->Q transition scales on DVE, emitted before the chunk's
    matmuls; transitioning/band banks matmul'd last in the chunk.
  - c=0 epilogue fused: per-bank copies chase the final matmuls; output
    leaves as two contiguous bf16 DMAs.
Host sums j-half core pairs, un-permutes rows, divides, applies elu.
"""

import ml_dtypes
import numpy as np

import concourse.bacc as bacc
import concourse.bass as bass
import concourse.mybir as mybir
import concourse.tile as tile
from concourse import bass_utils

F32 = mybir.dt.float32
BF16 = mybir.dt.bfloat16
FP16 = mybir.dt.float16
FP8 = mybir.dt.float8e4
OP = mybir.AluOpType

N = 8192
K = 256
F = 128
ALPHA = 0.2
NCORES = 8
MI = 2048
MJ = 4096
P = 128
NJ = MJ // P      # 32 j-chunks
NIT = MI // P     # 16 i-blocks
LAG = 5
GW = F + 1
SLAB = 8          # gq/gp slab = 8 chunks
W1 = 4            # es1b blocks carried in the meta blob
META_W = 2 * NJ + 2 * NIT + W1 * P   # fp16 units: es2m(f32) es1bt(f32) es1b p1

# m8 transfer groups, descending consumption order
M8_GROUPS = (
    [[31], [30, 29], [28, 27], [26, 25], [24, 23], [22, 21]]
    + [[20, 19, 18, 17], [16, 15, 14, 13], [12, 11, 10, 9], [8, 7, 6, 5],
       [4, 3, 2, 1], [0]]
)


def build_program(kq, kp):
    BW = max(1, max(p - q for p, q in zip(kp, kq)))
    assert max(kp[c] for c in range(24, NJ)) <= W1
    nc = bacc.Bacc("TRN2", target_bir_lowering=False)

    m8_d = nc.dram_tensor("m8", (P, NJ * MI), FP8, kind="ExternalInput")
    gg_d = nc.dram_tensor("gg", (P, 2 * NJ * GW), FP16, kind="ExternalInput")
    meta_d = nc.dram_tensor("meta", (P, META_W), FP16, kind="ExternalInput")
    es1bf_d = nc.dram_tensor("es1bf", (P, MI), FP16, kind="ExternalInput")
    out_d = nc.dram_tensor("out", (P, NIT * GW), BF16, kind="ExternalOutput")

    # per-block count of P-chunks (suffix in c); 0 -> no scale needed
    pP = [sum(1 for c in range(NJ) if kp[c] <= it) for it in range(NIT)]

    with tile.TileContext(nc) as tc:
        with (
            tc.tile_pool(name="consts", bufs=1) as consts,
            tc.tile_pool(name="adj", bufs=1) as adjp,
            tc.tile_pool(name="ggp", bufs=1) as ggp,
            tc.tile_pool(name="up", bufs=6) as up,
            tc.tile_pool(name="ntp", bufs=6) as ntp,
            tc.tile_pool(name="outp", bufs=1) as outp,
            tc.tile_pool(name="ps_acc", bufs=1, space="PSUM") as ps_acc,
        ):
            meta = consts.tile([P, META_W], FP16, tag="meta")
            es1bf = consts.tile([P, MI], FP16, tag="es1bf")
            es2m = meta[:, : 2 * NJ].bitcast(F32)          # [P, NJ]
            es1bt = meta[:, 2 * NJ : 2 * NJ + 2 * NIT].bitcast(F32)
            es1b_p1 = meta[:, 2 * NJ + 2 * NIT :]          # [P, W1*P]

            ggt = [
                ggp.tile([P, 2 * SLAB * GW], FP16, tag=f"gg{s}", name=f"gg{s}")
                for s in range(NJ // SLAB)
            ]
            m8t = [
                adjp.tile(
                    [P, len(g) * MI], FP8, tag=f"m8_{gi}", name=f"m8_{gi}"
                )
                for gi, g in enumerate(M8_GROUPS)
            ]

            accs = [
                ps_acc.tile([P, 512], F32, tag=f"acc{b}", name=f"acc{b}")
                for b in range(8)
            ]

            def acc_slice(it):
                return accs[it // 2][:, (it % 2) * 256 : (it % 2) * 256 + GW]

            # chunk -> (transfer index, offset-in-transfer)
            c2t = {}
            for gi, g in enumerate(M8_GROUPS):
                for ofs, cc in enumerate(sorted(g)):
                    c2t[cc] = (gi, ofs)

            def dma_m8(eng, gi):
                g = M8_GROUPS[gi]
                lo = min(g)
                eng.dma_start(
                    out=m8t[gi][:],
                    in_=m8_d[:, lo * MI : (lo + len(g)) * MI],
                )

            def dma_gg(eng, s):
                w = 2 * SLAB * GW
                eng.dma_start(
                    out=ggt[s][:], in_=gg_d[:, (3 - s) * w : (4 - s) * w]
                )

            # ---- DMA issue phase: everything up front, consumption order
            S, A = nc.sync, nc.scalar
            dma_m8(S, 0)                      # c31
            dma_gg(A, 3)
            dma_m8(S, 1)                      # c30,29
            A.dma_start(out=meta[:], in_=meta_d[:, :])
            dma_m8(S, 2)                      # c28,27
            A.dma_start(out=es1bf[:], in_=es1bf_d[:, :])
            dma_m8(S, 3)                      # c26,25
            dma_m8(A, 4)                      # c24,23
            dma_m8(S, 5)                      # c22,21
            dma_gg(A, 2)
            dma_m8(S, 6)                      # c20-17
            dma_m8(A, 7)                      # c16-13
            dma_gg(S, 1)
            dma_m8(A, 8)                      # c12-9
            dma_m8(S, 9)                      # c8-5
            dma_gg(A, 0)
            dma_m8(S, 10)                     # c4-1
            dma_m8(A, 11)                     # c0

            def m8_stat(c, it):
                gi, ofs = c2t[c]
                return m8t[gi][:, ofs * MI + it * P : ofs * MI + (it + 1) * P]

            def gq_mov(c):
                s = c // SLAB
                return ggt[s][:, (c % SLAB) * GW : (c % SLAB) * GW + GW]

            def gp_mov(c):
                s = c // SLAB
                o = (SLAB + c % SLAB) * GW
                return ggt[s][:, o : o + GW]

            pend = []
            scaled = [False] * NIT
            band_tiles = {}

            def phase_a(c):
                bw = kp[c] - kq[c]
                if bw > 0:
                    w = bw * P
                    u_t = up.tile([P, BW * P], FP16, tag="u_t")
                    if c >= 24:
                        src = es1b_p1[:, kq[c] * P : kp[c] * P]
                    else:
                        src = es1bf[:, kq[c] * P : kp[c] * P]
                    nc.vector.tensor_scalar(
                        out=u_t[:, :w],
                        in0=src,
                        scalar1=es2m[:, c : c + 1],
                        scalar2=1.0,
                        op0=OP.mult,
                        op1=OP.max,
                    )
                    n_t = ntp.tile([P, BW * P], FP16, tag="n_t")
                    gi, ofs = c2t[c]
                    nc.vector.tensor_tensor(
                        out=n_t[:, :w],
                        in0=u_t[:, :w],
                        in1=m8t[gi][
                            :, ofs * MI + kq[c] * P : ofs * MI + kp[c] * P
                        ],
                        op=OP.mult,
                    )
                    band_tiles[c] = n_t
                pend.append(c)

            res = outp.tile([P, NIT * GW], BF16, tag="res")

            def scale_acc(it):
                nc.vector.tensor_scalar(
                    out=acc_slice(it),
                    in0=acc_slice(it),
                    scalar1=es1bt[:, it : it + 1],
                    scalar2=None,
                    op0=OP.mult,
                )
                scaled[it] = True

            def phase_c():
                c = pend.pop(0)
                n_t = band_tiles.pop(c, None)
                last = c == 0
                trans = [
                    it
                    for it in range(NIT)
                    if it < kp[c] and pP[it] > 0 and not scaled[it]
                ]
                for it in trans:
                    scale_acc(it)
                # banks ordered: plain P/Q first, band-containing, then
                # transitioning last (hides scale/n_t latency); c=0 keeps
                # natural order so copies/out-DMA pipeline per half.
                def bank_key(b):
                    its = (2 * b, 2 * b + 1)
                    return (
                        any(it in trans for it in its),
                        any(kq[c] <= it < kp[c] for it in its),
                        b,
                    )

                banks = (
                    list(range(8)) if last else sorted(range(8), key=bank_key)
                )
                for b in banks:
                    for it in (2 * b, 2 * b + 1):
                        if it >= kp[c]:
                            stat = m8_stat(c, it)
                            mov = gp_mov(c)
                        elif it < kq[c]:
                            stat = m8_stat(c, it)
                            mov = gq_mov(c)
                        else:
                            stat = n_t[
                                :, (it - kq[c]) * P : (it - kq[c] + 1) * P
                            ]
                            mov = gq_mov(c)
                        nc.tensor.matmul(
                            acc_slice(it),
                            stat,
                            mov,
                            start=(c == NJ - 1 and it % 2 == 0),
                            stop=last,
                            skip_group_check=True,
                        )
                    if last:
                        for it in (2 * b, 2 * b + 1):
                            if pP[it] > 0 and not scaled[it]:
                                scale_acc(it)
                            dst = res[:, it * GW : (it + 1) * GW]
                            if it % 2 == 0:
                                nc.vector.tensor_copy(dst, acc_slice(it))
                            else:
                                nc.scalar.copy(dst, acc_slice(it))
                        if b == 3:
                            nc.sync.dma_start(
                                out=out_d[:, : 8 * GW], in_=res[:, : 8 * GW]
                            )
                        elif b == 7:
                            nc.scalar.dma_start(
                                out=out_d[:, 8 * GW :], in_=res[:, 8 * GW :]
                            )

            order = list(range(NJ - 1, -1, -1))
            for idx, c in enumerate(order):
                phase_a(c)
                if idx >= LAG:
                    phase_c()
            while pend:
                phase_c()

    nc.compile()
    return nc


def host_prepare(x, adj, W, a):
    h64 = x.astype(np.float64) @ W.astype(np.float64)
    s1 = h64 @ a[:F, 0].astype(np.float64)
    s2 = h64 @ a[F:, 0].astype(np.float64)
    es2a = np.exp(ALPHA * s2)
    es2m = np.exp((1.0 - ALPHA) * s2)
    es2f = np.exp(s2)
    g2 = np.empty((N, GW), np.float64)
    g2[:, :F] = h64
    g2[:, F] = 1.0
    gq = (g2 * es2a[:, None]).astype(np.float16)
    gp = (g2 * es2f[:, None]).astype(np.float16)
    es1b16 = np.exp((1.0 - ALPHA) * s1).astype(np.float16)

    isort = np.argsort(s1, kind="stable")
    ilists = [isort[sl::4] for sl in range(4)]
    jsort = np.argsort(s2, kind="stable")
    jlists = [jsort[h::2] for h in range(2)]

    maskT = adj.T > 0

    kq_all, kp_all = [], []
    for h in range(2):
        es2m_h = es2m[jlists[h]].astype(np.float32)
        cmax = es2m_h.reshape(NJ, P).max(axis=1)
        cmin = es2m_h.reshape(NJ, P).min(axis=1)
        for sl in range(4):
            e1 = es1b16[ilists[sl]].astype(np.float32).reshape(NIT, P)
            bmax = e1.max(axis=1)
            bmin = e1.min(axis=1)
            kq_all.append((bmax[None, :] * cmax[:, None] <= 1.0).sum(axis=1))
            # P-suffix count: blocks with bmin * es2m_chunk_min >= 1
            cnt = (bmin[None, :] * cmin[:, None] >= 1.0).sum(axis=1)
            kp_all.append(NIT - cnt)
    kq = np.minimum.reduce(kq_all).astype(int)
    kp = np.maximum.reduce(kp_all).astype(int)
    kp = np.maximum(kp, kq)  # band must be non-negative
    kq_l, kp_l = kq.tolist(), kp.tolist()

    in_maps = []
    for c in range(NCORES):
        sl = c % 4
        h = c // 4
        il, jl = ilists[sl], jlists[h]
        mT = maskT[np.ix_(jl, il)]
        # (P, NJ*MI): row p holds chunk-contiguous mask data
        m8 = np.ascontiguousarray(
            mT.reshape(NJ, P, MI).transpose(1, 0, 2).reshape(P, NJ * MI)
        ).astype(ml_dtypes.float8_e4m3)

        gqh = gq[jl].reshape(NJ, P, GW).transpose(1, 0, 2)   # [P, NJ, GW]
        gph = gp[jl].reshape(NJ, P, GW).transpose(1, 0, 2)
        # gg: per 8-chunk slab (desc order s3..s0): [gq_s | gp_s]
        gg = np.empty((P, 2 * NJ * GW), np.float16)
        w = 2 * SLAB * GW
        for s in range(4):
            base = (3 - s) * w
            gg[:, base : base + SLAB * GW] = gqh[
                :, s * SLAB : (s + 1) * SLAB
            ].reshape(P, SLAB * GW)
            gg[:, base + SLAB * GW : base + w] = gph[
                :, s * SLAB : (s + 1) * SLAB
            ].reshape(P, SLAB * GW)

        es2mh = np.ascontiguousarray(es2m[jl].reshape(NJ, P).T.astype(np.float32))
        es1bth = np.ascontiguousarray(
            es1b16[il].astype(np.float32).reshape(NIT, P).T
        )
        es1b_row = es1b16[il]                                 # [MI]
        meta_b = np.empty((P, 2 * META_W), np.uint8)
        meta_b[:, : 4 * NJ] = es2mh.view(np.uint8)
        meta_b[:, 4 * NJ : 4 * NJ + 4 * NIT] = es1bth.view(np.uint8)
        meta_b[:, 4 * NJ + 4 * NIT :] = np.broadcast_to(
            es1b_row[: W1 * P].view(np.uint8), (P, 2 * W1 * P)
        )
        es1bf = np.ascontiguousarray(
            np.broadcast_to(es1b_row, (P, MI))
        )
        in_maps.append(
            {
                "m8": m8,
                "gg": np.ascontiguousarray(gg),
                "meta": meta_b.view(np.float16),
                "es1bf": es1bf,
            }
        )
    return in_maps, kq_l, kp_l, ilists


_NC_CACHE = {}


def kernel(x, adj, W, a, _trace=False):
    x = np.asarray(x)
    adj = np.asarray(adj)
    W = np.asarray(W)
    a = np.asarray(a)

    in_maps, kq, kp, ilists = host_prepare(x, adj, W, a)
    key = (tuple(kq), tuple(kp))
    if key not in _NC_CACHE:
        _NC_CACHE.clear()
        _NC_CACHE[key] = build_program(kq, kp)
    nc = _NC_CACHE[key]
    res = bass_utils.run_bass_kernel_spmd(
        nc, in_maps, core_ids=list(range(NCORES)), trace=_trace
    )
    nd = np.empty((N, GW), np.float32)
    for sl in range(4):
        a0 = np.asarray(res.results[sl]["out"]).astype(np.float32)
        a1 = np.asarray(res.results[sl + 4]["out"]).astype(np.float32)
        both = (a0 + a1).reshape(P, NIT, GW).transpose(1, 0, 2).reshape(MI, GW)
        nd[ilists[sl]] = both
    hp = nd[:, :F] / nd[:, F : F + 1]
    out = np.where(hp > 0, hp, np.expm1(np.minimum(hp, 0.0))).astype(np.float32)
    if _trace:
        return out, res
    return out
